# revision 12
# baseline (speedup 1.0000x reference)
"""CapsuleLayer (dynamic routing) Trainium2 kernel — v2.

Sharding: in_units I=1024 split across 8 cores (128 each); W sharded on I.
Per-iteration s_j reduction completed with an AllReduce (fp32, 256 KB).

Key design vs v1 baseline:
  - Single fp16 for x and W (tolerance is 2e-2; fp16 end-to-end err ~4e-3).
    Halves HBM traffic (67 MB W per core) and uses 1 matmul per tile
    instead of 3 (fp16 hi/lo split).
  - u_hat kept RESIDENT in SBUF as fp16 (16.8 MB/core = 128 KiB/partition):
    no DRAM spill, routing passes do zero DMA.
  - Routing passes: agreement u.v via DVE mult(2x fp16) + segmented
    tensor_reduce; softmax batched over all groups in single instructions;
    c-expansion on the scalar engine (broadcast AP + per-partition 1/Z
    scale); c*u on DVE/GpSimd split; sum_i via delta-matmul accumulating
    all groups in PSUM.

Per-core layout: 128-partition tiles pack p_i=4 i-values x b=32 batch;
free axis = (j,d) = 2048. 32 group tiles cover I_CORE=128.
"""

import numpy as np

import concourse.bass as bass
import concourse.bacc as bacc
import concourse.mybir as mybir
from concourse import tile

AF = mybir.ActivationFunctionType
ALU = mybir.AluOpType
FP32 = mybir.dt.float32
FP16 = mybir.dt.float16

# Full-problem dims
B, I, C = 32, 1024, 128
J, D = 32, 64
N_CORES = 8


def build_kernel(nc, tc, dims, ins, outs):
    b, j, d, c = dims["B"], dims["J"], dims["D"], dims["C"]
    i_core = dims["I_CORE"]
    jd = j * d
    p_i = 128 // b                 # i's packed per 128-partition tile (4)
    groups = i_core // p_i         # group tiles per core (32)
    half = jd // 2                 # 1024
    nch = jd // 512                # fp32-out matmul chunks (4)
    replica_groups = [list(range(dims["N_CORES"]))]

    gps_agr = dims.get("GPS_AGR", 2)   # every nth group's agreement mult on GpSimd
    gps_w = dims.get("GPS_W", 2)       # every nth group's c*u mult on GpSimd
    ce_eng = dims.get("CE_ENG", "act")

    xT, Wt = ins["xT"], ins["Wt"]      # (G, C, P_I*B), (G, 2, C, P_I*JD/2)
    d_bM, d_Mb = ins["d_bM"], ins["d_Mb"]
    out = outs["out"]                  # (B, JD) fp16

    with (
        tc.tile_pool(name="const", bufs=1) as constp,
        tc.tile_pool(name="w", bufs=dims.get("WB", 3)) as wp,
        tc.tile_pool(name="x", bufs=2) as xp,
        tc.tile_pool(name="u", bufs=1) as up,
        tc.tile_pool(name="scr", bufs=dims.get("SCRB", 4)) as scrp,
        tc.tile_pool(name="big", bufs=1) as bigp,
        tc.tile_pool(name="small", bufs=1) as smp,
        tc.tile_pool(name="sq", bufs=1) as sqp,
        tc.tile_pool(name="psA", bufs=2, space="PSUM") as psA,
        tc.tile_pool(name="psS", bufs=1, space="PSUM") as psS,
        tc.tile_pool(name="dram", bufs=1, space="DRAM") as dram,
    ):
        dbM = constp.tile([b, 128], FP16)
        dMb = constp.tile([128, b], FP16)
        nc.sync.dma_start(dbM[:], d_bM[:])
        nc.sync.dma_start(dMb[:], d_Mb[:])

        ar_in = dram.tile([b, jd], FP32)
        ar_out = [dram.tile([b, jd], FP32, tag=f"ar_out{i}", name=f"ar_out{i}")
                  for i in range(3)]

        u_tiles = [up.tile([128, jd], FP16, tag=f"u{g}", name=f"u{g}")
                   for g in range(groups)]
        # bstate holds (g-major, j) agreement sums: shape (128, groups*j)
        bstate = bigp.tile([128, groups * j], FP32, tag="bst", name="bst")
        b2 = bigp.tile([128, groups * j], FP32, tag="b2", name="b2")
        eb = bigp.tile([128, groups * j], FP16, tag="eb", name="eb")
        bsub = bigp.tile([128, groups * j], FP16, tag="bsub", name="bsub")
        vrep = bigp.tile([128, jd], FP16, tag="vrep", name="vrep")
        nmx = smp.tile([128, groups], FP32, tag="nmx")
        zs = smp.tile([128, groups], FP32, tag="zs")
        rz = smp.tile([128, groups], FP32, tag="rz")
        s_sb = bigp.tile([b, jd], FP32, tag="s_sb", name="s_sb")
        s_red = bigp.tile([b, jd], FP32, tag="s_red", name="s_red")
        v_t = bigp.tile([b, jd], FP16, tag="v_t", name="v_t")
        n2 = sqp.tile([b, j], FP32, tag="n2")
        r0 = sqp.tile([b, j], FP32, tag="r0")
        rr = sqp.tile([b, j], FP32, tag="rr")
        dn = sqp.tile([b, j], FP32, tag="dn")

        # ---------------- Phase A: u_hat (+ s1 delta-MM) ----------------
        s1ps = psS.tile([b, jd], FP32, tag="sacc", name="s1ps")
        for g in range(groups):
            xg = xp.tile([c, p_i * b], FP16, tag="xg")
            nc.sync.dma_start(xg[:], xT[g, :, :])
            ug = u_tiles[g]
            for h in range(2):
                wgh = wp.tile([c, p_i * half], FP16, tag="wg")
                nc.sync.dma_start(wgh[:], Wt[g, h, :, :])
                ps = psA.tile([128, half], FP32, tag="psA")
                for k in range(p_i):
                    for n in range(2):
                        n0, n1 = n * 512, (n + 1) * 512
                        nc.tensor.matmul(
                            ps[k * b:(k + 1) * b, n0:n1],
                            lhsT=xg[:, k * b:(k + 1) * b],
                            rhs=wgh[:, k * half + n0:k * half + n1],
                            start=True, stop=True,
                            tile_position=(0, (k * b) % 128),
                        )
                nc.scalar.copy(out=ug[:, h * half:(h + 1) * half], in_=ps[:])
            for n in range(nch):
                n0, n1 = n * 512, (n + 1) * 512
                nc.tensor.matmul(s1ps[:, n0:n1], lhsT=dMb[:],
                                 rhs=ug[:, n0:n1],
                                 start=(g == 0), stop=(g == groups - 1))

        def all_reduce(src_ps, idx, scale):
            # src_ps: (b, jd) fp32 PSUM accumulator -> SBUF -> DRAM -> AR
            nc.scalar.mul(out=s_sb[:], in_=src_ps[:], mul=scale)
            nc.sync.dma_start(ar_in[:], s_sb[:])
            nc.gpsimd.collective_compute(
                "AllReduce", ALU.add,
                replica_groups=replica_groups,
                ins=[ar_in.opt()],
                outs=[ar_out[idx].opt()],
            )
            nc.sync.dma_start(s_red[:], ar_out[idx][:])
            return s_red

        def squash(s_t):
            # factor = n/(1+n^2), n = ||s[b,j,:]||; v = s * factor (fp16)
            # s can reach ~1e3, so square (s/64)^2 to stay in fp16 range:
            # n2' = n2/4096; n = 64*sqrt(n2'); factor = 32*(2*sqrt(n2'))/(1+4096*n2')
            sq = scrp.tile([b, jd], FP16, tag="scr")
            nc.scalar.activation(out=sq[:], in_=s_t[:], func=AF.Square,
                                 scale=1.0 / 64)
            nc.vector.tensor_reduce(
                out=n2[:, :, None],
                in_=sq[:].rearrange("p (j d) -> p j d", j=j),
                axis=mybir.AxisListType.X, op=ALU.add)
            nc.scalar.activation(out=r0[:], in_=n2[:], func=AF.Sqrt)
            nc.vector.reciprocal(out=rr[:], in_=r0[:])
            nc.vector.tensor_tensor(out=rr[:], in0=rr[:], in1=n2[:], op=ALU.mult)
            nc.vector.tensor_tensor(out=rr[:], in0=rr[:], in1=r0[:], op=ALU.add)
            nc.vector.tensor_scalar_mul(dn[:], n2[:], 4096.0)
            nc.vector.tensor_scalar_add(dn[:], dn[:], 1.0)
            nc.vector.reciprocal(out=dn[:], in_=dn[:])
            nc.vector.tensor_tensor(out=dn[:], in0=dn[:], in1=rr[:], op=ALU.mult)
            nc.vector.tensor_scalar_mul(dn[:], dn[:], 32.0)
            nc.vector.tensor_tensor(
                out=v_t[:].rearrange("p (j d) -> p j d", j=j),
                in0=s_t[:].rearrange("p (j d) -> p j d", j=j),
                in1=dn[:, :, None].to_broadcast((b, j, d)),
                op=ALU.mult)
            return v_t

        def replicate(v):
            # v (b, jd) fp16 -> vrep (128, jd) fp16 via delta matmul
            for h in range(2):
                psr = psA.tile([128, half], FP32, tag="psA")
                for n in range(2):
                    n0, n1 = n * 512, (n + 1) * 512
                    nc.tensor.matmul(psr[:, n0:n1], lhsT=dbM[:],
                                     rhs=v[:, h * half + n0:h * half + n1],
                                     start=True, stop=True)
                nc.scalar.copy(out=vrep[:, h * half:(h + 1) * half], in_=psr[:])
            return vrep

        dbg = dims.get("DEBUG")

        s_r = all_reduce(s1ps, 0, 1.0 / j)
        if dims.get("ONLY_A"):
            nc.scalar.copy(out=v_t[:], in_=s_r[:])
            nc.sync.dma_start(out[:], v_t[:])
            return
        v = squash(s_r)
        if dbg:
            nc.sync.dma_start(outs["dbg_s1"][:], s_red[:])
            nc.sync.dma_start(outs["dbg_v1"][:], v[:])
            nc.sync.dma_start(outs["dbg_u0"][:], u_tiles[0][:])
        vr = replicate(v)

        # ---------------- Routing passes 2 and 3 ----------------
        for it in range(2):
            btile = bstate if it == 0 else b2
            # agreement: b_inc[p, g, j] = sum_d u*g * vrep
            for g in range(groups):
                t = scrp.tile([128, jd], FP16, tag="scr")
                eng = nc.gpsimd if (gps_agr and g % gps_agr == 1) else nc.vector
                eng.tensor_tensor(out=t[:], in0=u_tiles[g][:], in1=vr[:],
                                  op=ALU.mult)
                nc.vector.tensor_reduce(
                    out=btile[:, g * j:(g + 1) * j, None],
                    in_=t[:].rearrange("p (j d) -> p j d", j=j),
                    axis=mybir.AxisListType.X, op=ALU.add)
            if dbg and it == 1:
                nc.sync.dma_start(outs["dbg_a3"][:], b2[:])
                nc.sync.dma_start(outs["dbg_vr2"][:], vr[:])
            if it == 1:
                nc.vector.tensor_tensor(out=b2[:], in0=b2[:], in1=bstate[:],
                                        op=ALU.add)
                if dbg:
                    nc.sync.dma_start(outs["dbg_b3"][:], b2[:])
            # softmax over j, batched over all groups
            nc.vector.tensor_reduce(
                out=nmx[:, :, None],
                in_=btile[:].rearrange("p (g j) -> p g j", g=groups),
                axis=mybir.AxisListType.X, op=ALU.max, negate=True)
            nc.vector.tensor_tensor(
                out=bsub[:].rearrange("p (g j) -> p g j", g=groups),
                in0=btile[:].rearrange("p (g j) -> p g j", g=groups),
                in1=nmx[:, :, None].to_broadcast((128, groups, j)),
                op=ALU.add)
            nc.scalar.activation(out=eb[:], in_=bsub[:], func=AF.Exp)
            nc.vector.tensor_reduce(
                out=zs[:, :, None],
                in_=eb[:].rearrange("p (g j) -> p g j", g=groups),
                axis=mybir.AxisListType.X, op=ALU.add)
            nc.vector.reciprocal(out=rz[:], in_=zs[:])
            # w = c * u; s += sum_i w via delta-MM (single strip, PSUM acc)
            sps = psS.tile([b, jd], FP32, tag="sacc", name=f"sps{it}")
            for g in range(groups):
                ce = scrp.tile([128, jd], FP16, tag="scr")
                ce_in = eb[:, g * j:(g + 1) * j, None].to_broadcast((128, j, d))
                ce_out = ce[:].rearrange("p (j d) -> p j d", j=j)
                if ce_eng == "act":
                    nc.scalar.mul(out=ce_out, in_=ce_in, mul=rz[:, g:g + 1])
                else:
                    eng0 = nc.gpsimd if ce_eng == "gpsimd" else nc.vector
                    eng0.tensor_scalar_mul(ce_out, ce_in, rz[:, g:g + 1])
                w = scrp.tile([128, jd], FP16, tag="scr")
                eng = nc.gpsimd if (gps_w and g % gps_w == 0) else nc.vector
                eng.tensor_tensor(out=w[:], in0=u_tiles[g][:], in1=ce[:],
                                  op=ALU.mult)
                if dbg and it == 0 and g == 0:
                    nc.sync.dma_start(outs["dbg_ce0"][:], ce[:])
                    nc.sync.dma_start(outs["dbg_w0"][:], w[:])
                for n in range(nch):
                    n0, n1 = n * 512, (n + 1) * 512
                    nc.tensor.matmul(sps[:, n0:n1], lhsT=dMb[:],
                                     rhs=w[:, n0:n1],
                                     start=(g == 0), stop=(g == groups - 1))
            if dbg and it == 0:
                nc.sync.dma_start(outs["dbg_b2"][:], bstate[:])
                nc.sync.dma_start(outs["dbg_eb"][:], eb[:])
                nc.sync.dma_start(outs["dbg_rz"][:], rz[:])
            if dbg and it == 1:
                nc.sync.dma_start(outs["dbg_eb3"][:], eb[:])
            s_r = all_reduce(sps, it + 1, 1.0)
            if dbg and it == 0:
                nc.sync.dma_start(outs["dbg_s2"][:], s_red[:])
            if dbg and it == 1:
                nc.sync.dma_start(outs["dbg_s3"][:], s_red[:])
            v = squash(s_r)
            if it == 0:
                vr = replicate(v)

        nc.sync.dma_start(out[:], v[:])


def _host_prep(x, W, n_cores, dims):
    """Shard + transpose inputs per core (fp16)."""
    b, j, d, c = dims["B"], dims["J"], dims["D"], dims["C"]
    i_core = dims["I_CORE"]
    p_i = 128 // b
    groups = i_core // p_i
    kc = n_cores
    jh = j // 2
    # xT: (kc, G, C, P_I*B) from x (B, I, C)
    xt = np.ascontiguousarray(
        x.reshape(b, kc, groups, p_i, c).transpose(1, 2, 4, 3, 0)
    ).reshape(kc, groups, c, p_i * b).astype(np.float16)
    # Wt: (kc, G, 2, C, P_I*JD/2) from W (J, I, D, C); half h = j in [h*16,(h+1)*16)
    wt = np.ascontiguousarray(
        W.reshape(2, jh, kc, groups, p_i, d, c).transpose(2, 3, 0, 6, 4, 1, 5)
    ).reshape(kc, groups, 2, c, p_i * jh * d).astype(np.float16)
    d_bM = np.tile(np.eye(b, dtype=np.float16), (1, p_i))      # (B, 128)
    d_Mb = np.ascontiguousarray(d_bM.T)                        # (128, B)
    in_maps = []
    for k in range(kc):
        in_maps.append({"xT": xt[k], "Wt": wt[k], "d_bM": d_bM, "d_Mb": d_Mb})
    return in_maps


def make_nc(dims):
    nc = bacc.Bacc("TRN2", target_bir_lowering=False, debug=False,
                   enable_asserts=False, num_devices=dims["N_CORES"])
    b, j, d, c = dims["B"], dims["J"], dims["D"], dims["C"]
    p_i = 128 // b
    groups = dims["I_CORE"] // p_i
    ins = {
        "xT": nc.dram_tensor("xT", [groups, c, p_i * b], FP16,
                             kind="ExternalInput").ap(),
        "Wt": nc.dram_tensor("Wt", [groups, 2, c, p_i * j * d // 2], FP16,
                             kind="ExternalInput").ap(),
        "d_bM": nc.dram_tensor("d_bM", [b, 128], FP16,
                               kind="ExternalInput").ap(),
        "d_Mb": nc.dram_tensor("d_Mb", [128, b], FP16,
                               kind="ExternalInput").ap(),
    }
    outs = {
        "out": nc.dram_tensor("out", [b, j * d], FP16,
                              kind="ExternalOutput").ap(),
    }
    if dims.get("DEBUG"):
        for nm, shape, dt in [
            ("dbg_s1", [b, j * d], FP32), ("dbg_v1", [b, j * d], FP16),
            ("dbg_u0", [128, j * d], FP16), ("dbg_ce0", [128, j * d], FP16),
            ("dbg_w0", [128, j * d], FP16), ("dbg_b2", [128, groups * j], FP32),
            ("dbg_eb", [128, groups * j], FP16), ("dbg_rz", [128, groups], FP32),
            ("dbg_s2", [b, j * d], FP32), ("dbg_s3", [b, j * d], FP32),
            ("dbg_a3", [128, groups * j], FP32),
            ("dbg_b3", [128, groups * j], FP32),
            ("dbg_vr2", [128, j * d], FP16),
            ("dbg_eb3", [128, groups * j], FP16),
        ]:
            outs[nm] = nc.dram_tensor(nm, shape, dt,
                                      kind="ExternalOutput").ap()
    with tile.TileContext(nc) as tc:
        build_kernel(nc, tc, dims, ins, outs)
    nc.compile()
    return nc


_NC_CACHE = {}


def _build_runner(nc, n_cores):
    """Mirror of bass2jax.run_bass_via_pjrt multi-core tail, returning the
    jitted sharded callable so callers can re-invoke with device-resident
    inputs for timing."""
    import jax
    from jax.sharding import Mesh, PartitionSpec
    from jax.experimental.shard_map import shard_map
    import concourse.mybir as mb
    from concourse.bass2jax import (_bass_exec_p, install_neuronx_cc_hook,
                                    partition_id_tensor)
    install_neuronx_cc_hook()
    partition_name = (nc.partition_id_tensor.name
                      if nc.partition_id_tensor else None)
    in_names, out_names, out_avals, zero_outs = [], [], [], []
    for alloc in nc.m.functions[0].allocations:
        if not isinstance(alloc, mb.MemoryLocationSet):
            continue
        name = alloc.memorylocations[0].name
        if alloc.kind == "ExternalInput":
            if name != partition_name:
                in_names.append(name)
        elif alloc.kind == "ExternalOutput":
            shape = tuple(alloc.tensor_shape)
            dtype = mb.dt.np(alloc.dtype)
            out_avals.append(jax.core.ShapedArray(shape, dtype))
            zero_outs.append(np.zeros(shape, dtype))
            out_names.append(name)
    n_params = len(in_names)
    n_outs = len(out_avals)
    all_in_names = list(in_names) + list(out_names)
    if partition_name is not None:
        all_in_names.append(partition_name)
    donate = tuple(range(n_params, n_params + n_outs))

    def _body(*args):
        operands = list(args)
        if partition_name is not None:
            operands.append(partition_id_tensor())
        return tuple(_bass_exec_p.bind(
            *operands, out_avals=tuple(out_avals), in_names=tuple(all_in_names),
            out_names=tuple(out_names), lowering_input_output_aliases=(),
            sim_require_finite=True, sim_require_nnan=True, nc=nc))

    devices = jax.devices()[:n_cores]
    mesh = Mesh(np.asarray(devices), ("core",))
    in_specs = (PartitionSpec("core"),) * (n_params + n_outs)
    out_specs = (PartitionSpec("core"),) * n_outs
    fn = jax.jit(shard_map(_body, mesh=mesh, in_specs=in_specs,
                           out_specs=out_specs, check_rep=False),
                 donate_argnums=donate, keep_unused=True)
    return {"fn": fn, "in_names": in_names, "out_names": out_names,
            "out_avals": out_avals, "zero_outs": zero_outs, "mesh": mesh,
            "n_params": n_params}


EXTRA_DIMS = {}


def _get_runner():
    dims = {"B": B, "J": J, "D": D, "C": C, "I_CORE": I // N_CORES,
            "N_CORES": N_CORES}
    dims.update(EXTRA_DIMS)
    if "full" not in _NC_CACHE:
        nc = make_nc(dims)
        _NC_CACHE["full"] = (nc, _build_runner(nc, N_CORES), dims)
    return _NC_CACHE["full"]


def _concat_inputs(runner, in_maps, n_cores):
    return [np.concatenate([np.asarray(in_maps[c][name])
                            for c in range(n_cores)], axis=0)
            for name in runner["in_names"]]


def _concat_zeros(runner, n_cores):
    return [np.zeros((n_cores * z.shape[0], *z.shape[1:]), z.dtype)
            for z in runner["zero_outs"]]


def kernel(x, W):
    nc, runner, dims = _get_runner()
    in_maps = _host_prep(np.asarray(x, np.float32), np.asarray(W, np.float32),
                         N_CORES, dims)
    concat_in = _concat_inputs(runner, in_maps, N_CORES)
    out_arrs = runner["fn"](*concat_in, *_concat_zeros(runner, N_CORES))
    idx = runner["out_names"].index("out")
    aval = runner["out_avals"][idx]
    out = np.asarray(out_arrs[idx]).reshape(N_CORES, *aval.shape)[0]
    return out.reshape(B, J, D).astype(np.float32)


# revision 17
# speedup vs baseline: 1.2145x; 1.2145x over previous
"""CapsuleLayer (dynamic routing) Trainium2 kernel — v2.

Sharding: in_units I=1024 split across 8 cores (128 each); W sharded on I.
Per-iteration s_j reduction completed with an AllReduce (fp32, 256 KB).

Key design vs v1 baseline:
  - Single fp16 for x and W (tolerance is 2e-2; fp16 end-to-end err ~4e-3).
    Halves HBM traffic (67 MB W per core) and uses 1 matmul per tile
    instead of 3 (fp16 hi/lo split).
  - u_hat kept RESIDENT in SBUF as fp16 (16.8 MB/core = 128 KiB/partition):
    no DRAM spill, routing passes do zero DMA.
  - Routing passes: agreement u.v via DVE mult(2x fp16) + segmented
    tensor_reduce; softmax batched over all groups in single instructions;
    c-expansion on the scalar engine (broadcast AP + per-partition 1/Z
    scale); c*u on DVE/GpSimd split; sum_i via delta-matmul accumulating
    all groups in PSUM.

Per-core layout: 128-partition tiles pack p_i=4 i-values x b=32 batch;
free axis = (j,d) = 2048. 32 group tiles cover I_CORE=128.
"""

import numpy as np

import concourse.bass as bass
import concourse.bacc as bacc
import concourse.mybir as mybir
from concourse import tile

AF = mybir.ActivationFunctionType
ALU = mybir.AluOpType
FP32 = mybir.dt.float32
FP16 = mybir.dt.float16

# Full-problem dims
B, I, C = 32, 1024, 128
J, D = 32, 64
N_CORES = 8


def build_kernel(nc, tc, dims, ins, outs):
    b, j, d, c = dims["B"], dims["J"], dims["D"], dims["C"]
    i_core = dims["I_CORE"]
    jd = j * d
    p_i = 128 // b                 # i's packed per 128-partition tile (4)
    groups = i_core // p_i         # group tiles per core (32)
    half = jd // 2                 # 1024
    nch = jd // 512                # fp32-out matmul chunks (4)
    replica_groups = [list(range(dims["N_CORES"]))]

    gps_agr = dims.get("GPS_AGR", 2)   # every nth group's agreement mult on GpSimd
    gps_w = dims.get("GPS_W", 0)       # every nth group's c*u mult on GpSimd
    ce_eng = dims.get("CE_ENG", "act")

    xT, Wt = ins["xT"], ins["Wt"]      # (G, C, P_I*B), (G, 2, C, P_I*JD/2)
    d_bM, d_Mb = ins["d_bM"], ins["d_Mb"]
    out = outs["out"]                  # (B, JD) fp16

    with (
        tc.tile_pool(name="const", bufs=1) as constp,
        tc.tile_pool(name="w", bufs=dims.get("WB", 2)) as wp,
        tc.tile_pool(name="x", bufs=dims.get("XB", 1)) as xp,
        tc.tile_pool(name="u", bufs=1) as up,
        tc.tile_pool(name="scr", bufs=dims.get("SCRB", 4)) as scrp,
        tc.tile_pool(name="big", bufs=1) as bigp,
        tc.tile_pool(name="small", bufs=1) as smp,
        tc.tile_pool(name="sq", bufs=1) as sqp,
        tc.tile_pool(name="psA", bufs=2, space="PSUM") as psA,
        tc.tile_pool(name="psS", bufs=1, space="PSUM") as psS,
        tc.tile_pool(name="dram", bufs=1, space="DRAM") as dram,
    ):
        dbM = constp.tile([b, 128], FP16)
        dMb = constp.tile([128, b], FP16)
        nc.sync.dma_start(dbM[:], d_bM[:])
        nc.sync.dma_start(dMb[:], d_Mb[:])

        ar_in = dram.tile([b, jd], FP16)
        ar_out = [dram.tile([b, jd], FP16, tag=f"ar_out{i}", name=f"ar_out{i}")
                  for i in range(3)]

        u_tiles = [up.tile([128, jd], FP16, tag=f"u{g}", name=f"u{g}")
                   for g in range(groups)]
        # bstate holds (g-major, j) agreement sums: shape (128, groups*j)
        bstate = bigp.tile([128, groups * j], FP32, tag="bst", name="bst")
        b2 = bigp.tile([128, groups * j], FP32, tag="b2", name="b2")
        eb = bigp.tile([128, groups * j], FP16, tag="eb", name="eb")
        vrep = bigp.tile([128, jd], FP16, tag="vrep", name="vrep")
        nmx = smp.tile([128, groups], FP32, tag="nmx")
        zs = smp.tile([128, groups], FP32, tag="zs")
        rz = smp.tile([128, groups], FP32, tag="rz")
        s_sb = bigp.tile([b, jd], FP16, tag="s_sb", name="s_sb")
        s_red = bigp.tile([b, jd], FP16, tag="s_red", name="s_red")
        v_t = bigp.tile([b, jd], FP16, tag="v_t", name="v_t")
        n2 = sqp.tile([b, j], FP32, tag="n2")
        r0 = sqp.tile([b, j], FP32, tag="r0")
        rr = sqp.tile([b, j], FP32, tag="rr")
        dn = sqp.tile([b, j], FP32, tag="dn")

        # ---------------- Phase A: u_hat (+ s1 delta-MM) ----------------
        s1ps = psS.tile([b, jd], FP32, tag="sacc", name="s1ps")
        xgs = {}
        for g in range(groups):
            if g % 8 == 0:
                xc = xp.tile([c, 8 * p_i * b], FP16, tag="xg")
                nc.sync.dma_start(xc[:], xT[g // 8, :, :])
                xgs = {"t": xc, "g0": g}
            xg = xgs["t"]
            xoff = (g - xgs["g0"]) * p_i * b
            wg = wp.tile([c, p_i * jd], FP16, tag="wg")
            nc.sync.dma_start(wg[:], Wt[g, :, :])
            ug = u_tiles[g]
            for h in range(2):
                ps = psA.tile([128, half], FP32, tag="psA")
                for k in range(p_i):
                    for n in range(2):
                        n0, n1 = n * 512, (n + 1) * 512
                        c0 = k * jd + h * half
                        nc.tensor.matmul(
                            ps[k * b:(k + 1) * b, n0:n1],
                            lhsT=xg[:, xoff + k * b:xoff + (k + 1) * b],
                            rhs=wg[:, c0 + n0:c0 + n1],
                            start=True, stop=True,
                            tile_position=(0, (k * b) % 128),
                        )
                nc.scalar.copy(out=ug[:, h * half:(h + 1) * half], in_=ps[:])
            for n in range(nch):
                n0, n1 = n * 512, (n + 1) * 512
                nc.tensor.matmul(s1ps[:, n0:n1], lhsT=dMb[:],
                                 rhs=ug[:, n0:n1],
                                 start=(g == 0), stop=(g == groups - 1))

        def all_reduce(src_ps, idx, scale):
            # src_ps: (b, jd) fp32 PSUM accumulator -> SBUF -> DRAM -> AR
            nc.scalar.mul(out=s_sb[:], in_=src_ps[:], mul=scale)
            nc.sync.dma_start(ar_in[:], s_sb[:])
            nc.gpsimd.collective_compute(
                "AllReduce", ALU.add,
                replica_groups=replica_groups,
                ins=[ar_in.opt()],
                outs=[ar_out[idx].opt()],
            )
            nc.sync.dma_start(s_red[:], ar_out[idx][:])
            return s_red

        def squash(s_t):
            # factor = n/(1+n^2), n = ||s[b,j,:]||; v = s * factor (fp16)
            # s can reach ~1e3, so square (s/64)^2 to stay in fp16 range:
            # n2' = n2/4096; n = 64*sqrt(n2'); factor = 32*(2*sqrt(n2'))/(1+4096*n2')
            sq = scrp.tile([b, jd], FP16, tag="scr")
            nc.scalar.activation(out=sq[:], in_=s_t[:], func=AF.Square,
                                 scale=1.0 / 64)
            nc.vector.tensor_reduce(
                out=n2[:, :, None],
                in_=sq[:].rearrange("p (j d) -> p j d", j=j),
                axis=mybir.AxisListType.X, op=ALU.add)
            nc.scalar.activation(out=r0[:], in_=n2[:], func=AF.Sqrt)
            nc.vector.reciprocal(out=rr[:], in_=r0[:])
            nc.vector.tensor_tensor(out=rr[:], in0=rr[:], in1=n2[:], op=ALU.mult)
            nc.vector.tensor_tensor(out=rr[:], in0=rr[:], in1=r0[:], op=ALU.add)
            nc.vector.tensor_scalar_mul(dn[:], n2[:], 4096.0)
            nc.vector.tensor_scalar_add(dn[:], dn[:], 1.0)
            nc.vector.reciprocal(out=dn[:], in_=dn[:])
            nc.vector.tensor_tensor(out=dn[:], in0=dn[:], in1=rr[:], op=ALU.mult)
            nc.vector.tensor_scalar_mul(dn[:], dn[:], 32.0)
            nc.vector.tensor_tensor(
                out=v_t[:].rearrange("p (j d) -> p j d", j=j),
                in0=s_t[:].rearrange("p (j d) -> p j d", j=j),
                in1=dn[:, :, None].to_broadcast((b, j, d)),
                op=ALU.mult)
            return v_t

        def replicate(v):
            # v (b, jd) fp16 -> vrep (128, jd) fp16 via delta matmul
            for h in range(2):
                psr = psA.tile([128, half], FP32, tag="psA")
                for n in range(2):
                    n0, n1 = n * 512, (n + 1) * 512
                    nc.tensor.matmul(psr[:, n0:n1], lhsT=dbM[:],
                                     rhs=v[:, h * half + n0:h * half + n1],
                                     start=True, stop=True)
                nc.scalar.copy(out=vrep[:, h * half:(h + 1) * half], in_=psr[:])
            return vrep

        dbg = dims.get("DEBUG")

        s_r = all_reduce(s1ps, 0, 1.0 / j)
        if dims.get("ONLY_A"):
            nc.scalar.copy(out=v_t[:], in_=s_r[:])
            nc.sync.dma_start(out[:], v_t[:])
            return
        v = squash(s_r)
        if dbg:
            nc.sync.dma_start(outs["dbg_s1"][:], s_red[:])
            nc.sync.dma_start(outs["dbg_v1"][:], v[:])
            nc.sync.dma_start(outs["dbg_u0"][:], u_tiles[0][:])
        vr = replicate(v)

        # ---------------- Routing passes 2 and 3 ----------------
        # Software-pipelined blocks of BG groups: emission order
        # A(0) S(0) A(1) W(0) S(1) A(2) W(1) ... so DVE runs block b+1's
        # agreement while ACT expands block b's c and PE runs its delta-MMs.
        BG = dims.get("BG", 8)
        nblk = groups // BG

        for it in range(2):
            btile = bstate if it == 0 else b2
            # subtract target: reuse the other b tile (free at that point)
            subt = b2 if it == 0 else bstate
            sps = psS.tile([b, jd], FP32, tag="sacc", name=f"sps{it}")

            def stage_agree(bi):
                for g in range(bi * BG, (bi + 1) * BG):
                    t = scrp.tile([128, jd], FP16, tag="scr")
                    eng = (nc.gpsimd if (gps_agr and g % gps_agr == 1)
                           else nc.vector)
                    eng.tensor_tensor(out=t[:], in0=u_tiles[g][:], in1=vr[:],
                                      op=ALU.mult)
                    nc.vector.tensor_reduce(
                        out=btile[:, g * j:(g + 1) * j, None],
                        in_=t[:].rearrange("p (j d) -> p j d", j=j),
                        axis=mybir.AxisListType.X, op=ALU.add)

            def stage_softmax(bi):
                sl = slice(bi * BG * j, (bi + 1) * BG * j)
                slg = slice(bi * BG, (bi + 1) * BG)
                if it == 1:
                    nc.vector.tensor_tensor(out=b2[:, sl], in0=b2[:, sl],
                                            in1=bstate[:, sl], op=ALU.add)
                nc.vector.tensor_reduce(
                    out=nmx[:, slg, None],
                    in_=btile[:, sl].rearrange("p (g j) -> p g j", g=BG),
                    axis=mybir.AxisListType.X, op=ALU.max, negate=True)
                nc.vector.tensor_tensor(
                    out=subt[:, sl].rearrange("p (g j) -> p g j", g=BG),
                    in0=btile[:, sl].rearrange("p (g j) -> p g j", g=BG),
                    in1=nmx[:, slg, None].to_broadcast((128, BG, j)),
                    op=ALU.add)
                nc.scalar.activation(out=eb[:, sl], in_=subt[:, sl],
                                     func=AF.Exp)
                nc.vector.tensor_reduce(
                    out=zs[:, slg, None],
                    in_=eb[:, sl].rearrange("p (g j) -> p g j", g=BG),
                    axis=mybir.AxisListType.X, op=ALU.add)
                nc.vector.reciprocal(out=rz[:, slg], in_=zs[:, slg])

            def stage_w(bi):
                for g in range(bi * BG, (bi + 1) * BG):
                    ce = scrp.tile([128, jd], FP16, tag="scr")
                    ce_in = eb[:, g * j:(g + 1) * j, None].to_broadcast(
                        (128, j, d))
                    ce_out = ce[:].rearrange("p (j d) -> p j d", j=j)
                    if ce_eng == "act":
                        nc.scalar.mul(out=ce_out, in_=ce_in,
                                      mul=rz[:, g:g + 1])
                    else:
                        eng0 = (nc.gpsimd if ce_eng == "gpsimd"
                                else nc.vector)
                        eng0.tensor_scalar_mul(ce_out, ce_in, rz[:, g:g + 1])
                    w = scrp.tile([128, jd], FP16, tag="scr")
                    eng = (nc.gpsimd if (gps_w and g % gps_w == 0)
                           else nc.vector)
                    eng.tensor_tensor(out=w[:], in0=u_tiles[g][:], in1=ce[:],
                                      op=ALU.mult)
                    for n in range(nch):
                        n0, n1 = n * 512, (n + 1) * 512
                        nc.tensor.matmul(sps[:, n0:n1], lhsT=dMb[:],
                                         rhs=w[:, n0:n1],
                                         start=(g == 0),
                                         stop=(g == groups - 1))

            stage_agree(0)
            stage_softmax(0)
            for bi in range(1, nblk):
                stage_agree(bi)
                stage_w(bi - 1)
                stage_softmax(bi)
            stage_w(nblk - 1)

            s_r = all_reduce(sps, it + 1, 1.0)
            v = squash(s_r)
            if it == 0:
                vr = replicate(v)

        nc.sync.dma_start(out[:], v[:])


def _host_prep(x, W, n_cores, dims):
    """Shard + transpose inputs per core (fp16)."""
    b, j, d, c = dims["B"], dims["J"], dims["D"], dims["C"]
    i_core = dims["I_CORE"]
    p_i = 128 // b
    groups = i_core // p_i
    kc = n_cores
    jh = j // 2
    # xT: (kc, G/8, C, 8*P_I*B) from x (B, I, C)
    xt = np.ascontiguousarray(
        x.reshape(b, kc, groups // 8, 8, p_i, c).transpose(1, 2, 5, 3, 4, 0)
    ).reshape(kc, groups // 8, c, 8 * p_i * b).astype(np.float16)
    # Wt: (kc, G, C, P_I*JD) from W (J, I, D, C); cols = (k, j, d)
    wt = np.ascontiguousarray(
        W.reshape(j, kc, groups, p_i, d, c).transpose(1, 2, 5, 3, 0, 4)
    ).reshape(kc, groups, c, p_i * j * d).astype(np.float16)
    d_bM = np.tile(np.eye(b, dtype=np.float16), (1, p_i))      # (B, 128)
    d_Mb = np.ascontiguousarray(d_bM.T)                        # (128, B)
    in_maps = []
    for k in range(kc):
        in_maps.append({"xT": xt[k], "Wt": wt[k], "d_bM": d_bM, "d_Mb": d_Mb})
    return in_maps


def make_nc(dims):
    nc = bacc.Bacc("TRN2", target_bir_lowering=False, debug=False,
                   enable_asserts=False, num_devices=dims["N_CORES"])
    b, j, d, c = dims["B"], dims["J"], dims["D"], dims["C"]
    p_i = 128 // b
    groups = dims["I_CORE"] // p_i
    ins = {
        "xT": nc.dram_tensor("xT", [groups // 8, c, 8 * p_i * b], FP16,
                             kind="ExternalInput").ap(),
        "Wt": nc.dram_tensor("Wt", [groups, c, p_i * j * d], FP16,
                             kind="ExternalInput").ap(),
        "d_bM": nc.dram_tensor("d_bM", [b, 128], FP16,
                               kind="ExternalInput").ap(),
        "d_Mb": nc.dram_tensor("d_Mb", [128, b], FP16,
                               kind="ExternalInput").ap(),
    }
    outs = {
        "out": nc.dram_tensor("out", [b, j * d], FP16,
                              kind="ExternalOutput").ap(),
    }
    if dims.get("DEBUG"):
        for nm, shape, dt in [
            ("dbg_s1", [b, j * d], FP16), ("dbg_v1", [b, j * d], FP16),
            ("dbg_u0", [128, j * d], FP16),
        ]:
            outs[nm] = nc.dram_tensor(nm, shape, dt,
                                      kind="ExternalOutput").ap()
    with tile.TileContext(nc) as tc:
        build_kernel(nc, tc, dims, ins, outs)
    nc.compile()
    return nc


_NC_CACHE = {}


def _build_runner(nc, n_cores):
    """Mirror of bass2jax.run_bass_via_pjrt multi-core tail, returning the
    jitted sharded callable so callers can re-invoke with device-resident
    inputs for timing."""
    import jax
    from jax.sharding import Mesh, PartitionSpec
    from jax.experimental.shard_map import shard_map
    import concourse.mybir as mb
    from concourse.bass2jax import (_bass_exec_p, install_neuronx_cc_hook,
                                    partition_id_tensor)
    install_neuronx_cc_hook()
    partition_name = (nc.partition_id_tensor.name
                      if nc.partition_id_tensor else None)
    in_names, out_names, out_avals, zero_outs = [], [], [], []
    for alloc in nc.m.functions[0].allocations:
        if not isinstance(alloc, mb.MemoryLocationSet):
            continue
        name = alloc.memorylocations[0].name
        if alloc.kind == "ExternalInput":
            if name != partition_name:
                in_names.append(name)
        elif alloc.kind == "ExternalOutput":
            shape = tuple(alloc.tensor_shape)
            dtype = mb.dt.np(alloc.dtype)
            out_avals.append(jax.core.ShapedArray(shape, dtype))
            zero_outs.append(np.zeros(shape, dtype))
            out_names.append(name)
    n_params = len(in_names)
    n_outs = len(out_avals)
    all_in_names = list(in_names) + list(out_names)
    if partition_name is not None:
        all_in_names.append(partition_name)
    donate = tuple(range(n_params, n_params + n_outs))

    def _body(*args):
        operands = list(args)
        if partition_name is not None:
            operands.append(partition_id_tensor())
        return tuple(_bass_exec_p.bind(
            *operands, out_avals=tuple(out_avals), in_names=tuple(all_in_names),
            out_names=tuple(out_names), lowering_input_output_aliases=(),
            sim_require_finite=True, sim_require_nnan=True, nc=nc))

    devices = jax.devices()[:n_cores]
    mesh = Mesh(np.asarray(devices), ("core",))
    in_specs = (PartitionSpec("core"),) * (n_params + n_outs)
    out_specs = (PartitionSpec("core"),) * n_outs
    fn = jax.jit(shard_map(_body, mesh=mesh, in_specs=in_specs,
                           out_specs=out_specs, check_rep=False),
                 donate_argnums=donate, keep_unused=True)
    return {"fn": fn, "in_names": in_names, "out_names": out_names,
            "out_avals": out_avals, "zero_outs": zero_outs, "mesh": mesh,
            "n_params": n_params}


EXTRA_DIMS = {}


def _get_runner():
    dims = {"B": B, "J": J, "D": D, "C": C, "I_CORE": I // N_CORES,
            "N_CORES": N_CORES}
    dims.update(EXTRA_DIMS)
    if "full" not in _NC_CACHE:
        nc = make_nc(dims)
        _NC_CACHE["full"] = (nc, _build_runner(nc, N_CORES), dims)
    return _NC_CACHE["full"]


def _concat_inputs(runner, in_maps, n_cores):
    return [np.concatenate([np.asarray(in_maps[c][name])
                            for c in range(n_cores)], axis=0)
            for name in runner["in_names"]]


def _concat_zeros(runner, n_cores):
    return [np.zeros((n_cores * z.shape[0], *z.shape[1:]), z.dtype)
            for z in runner["zero_outs"]]


def kernel(x, W):
    nc, runner, dims = _get_runner()
    in_maps = _host_prep(np.asarray(x, np.float32), np.asarray(W, np.float32),
                         N_CORES, dims)
    concat_in = _concat_inputs(runner, in_maps, N_CORES)
    out_arrs = runner["fn"](*concat_in, *_concat_zeros(runner, N_CORES))
    idx = runner["out_names"].index("out")
    aval = runner["out_avals"][idx]
    out = np.asarray(out_arrs[idx]).reshape(N_CORES, *aval.shape)[0]
    return out.reshape(B, J, D).astype(np.float32)


# revision 18
# speedup vs baseline: 2.3083x; 1.9006x over previous
"""CapsuleLayer (dynamic routing) Trainium2 kernel.

Sharding: in_units I=1024 split across 8 cores (128 each); W sharded on I;
the per-iteration s_j = sum_i c_ij*u_hat reduction is completed with a
fp16 AllReduce (128 KB) per routing iteration.

Design:
  - Single fp16 for x and W (tolerance 2e-2; fp16 end-to-end err ~5.6e-3;
    bf16/fp8 fail — routing amplifies u_hat quantization ~6x). Halves HBM
    traffic (67 MB W per core streams at the DMA roofline, ~190 us).
  - u_hat resident in SBUF as fp16 (16.8 MB/core = 128 KiB/partition): no
    DRAM spill; routing passes do zero HBM traffic.
  - Phase A: per group tile (p=4 i's x b=32, free=(j,d)=2048), one fp16
    matmul per (i, 512-chunk) packed 4-up via tile_position col strips;
    PSUM fp32 -> fp16 SBUF copies on ScalarE; s1 accumulated with delta
    matmuls (128->32 partition fold) into a persistent PSUM tile.
  - Routing passes software-pipelined in blocks of 8 groups
    (A(b+1) emitted before W(b)): DVE does agreement mult (2x fp16) +
    segmented tensor_reduce (the 1x bottleneck) + c*u mult; GpSimd takes
    half the agreement mults; ScalarE expands c (broadcast AP * 1/Z
    per-partition scale) and evacuates PSUM; PE runs the delta-MM chain.
  - squash squares s/64 to stay in fp16 range (|s| reaches ~700).
"""

import numpy as np

import concourse.bass as bass
import concourse.bacc as bacc
import concourse.mybir as mybir
from concourse import tile

AF = mybir.ActivationFunctionType
ALU = mybir.AluOpType
FP32 = mybir.dt.float32
FP16 = mybir.dt.float16

# Full-problem dims
B, I, C = 32, 1024, 128
J, D = 32, 64
N_CORES = 8


def build_kernel(nc, tc, dims, ins, outs):
    b, j, d, c = dims["B"], dims["J"], dims["D"], dims["C"]
    i_core = dims["I_CORE"]
    jd = j * d
    p_i = 128 // b                 # i's packed per 128-partition tile (4)
    groups = i_core // p_i         # group tiles per core (32)
    half = jd // 2                 # 1024
    nch = jd // 512                # fp32-out matmul chunks (4)
    replica_groups = [list(range(dims["N_CORES"]))]

    gps_agr = dims.get("GPS_AGR", 2)   # every nth group's agreement mult on GpSimd
    gps_w = dims.get("GPS_W", 0)       # every nth group's c*u mult on GpSimd
    ce_eng = dims.get("CE_ENG", "act")

    xT, Wt = ins["xT"], ins["Wt"]      # (G, C, P_I*B), (G, 2, C, P_I*JD/2)
    d_bM, d_Mb = ins["d_bM"], ins["d_Mb"]
    out = outs["out"]                  # (B, JD) fp16

    with (
        tc.tile_pool(name="const", bufs=1) as constp,
        tc.tile_pool(name="w", bufs=dims.get("WB", 2)) as wp,
        tc.tile_pool(name="x", bufs=dims.get("XB", 1)) as xp,
        tc.tile_pool(name="u", bufs=1) as up,
        tc.tile_pool(name="scr", bufs=dims.get("SCRB", 4)) as scrp,
        tc.tile_pool(name="big", bufs=1) as bigp,
        tc.tile_pool(name="small", bufs=1) as smp,
        tc.tile_pool(name="sq", bufs=1) as sqp,
        tc.tile_pool(name="psA", bufs=2, space="PSUM") as psA,
        tc.tile_pool(name="psS", bufs=1, space="PSUM") as psS,
        tc.tile_pool(name="dram", bufs=1, space="DRAM") as dram,
    ):
        dbM = constp.tile([b, 128], FP16)
        dMb = constp.tile([128, b], FP16)
        nc.sync.dma_start(dbM[:], d_bM[:])
        nc.sync.dma_start(dMb[:], d_Mb[:])

        ar_in = dram.tile([b, jd], FP16)
        ar_out = [dram.tile([b, jd], FP16, tag=f"ar_out{i}", name=f"ar_out{i}")
                  for i in range(3)]

        u_tiles = [up.tile([128, jd], FP16, tag=f"u{g}", name=f"u{g}")
                   for g in range(groups)]
        # bstate holds (g-major, j) agreement sums: shape (128, groups*j)
        bstate = bigp.tile([128, groups * j], FP32, tag="bst", name="bst")
        b2 = bigp.tile([128, groups * j], FP32, tag="b2", name="b2")
        eb = bigp.tile([128, groups * j], FP16, tag="eb", name="eb")
        vrep = bigp.tile([128, jd], FP16, tag="vrep", name="vrep")
        nmx = smp.tile([128, groups], FP32, tag="nmx")
        zs = smp.tile([128, groups], FP32, tag="zs")
        rz = smp.tile([128, groups], FP32, tag="rz")
        s_sb = bigp.tile([b, jd], FP16, tag="s_sb", name="s_sb")
        s_red = bigp.tile([b, jd], FP16, tag="s_red", name="s_red")
        v_t = bigp.tile([b, jd], FP16, tag="v_t", name="v_t")
        n2 = sqp.tile([b, j], FP32, tag="n2")
        r0 = sqp.tile([b, j], FP32, tag="r0")
        rr = sqp.tile([b, j], FP32, tag="rr")
        dn = sqp.tile([b, j], FP32, tag="dn")

        # ---------------- Phase A: u_hat (+ s1 delta-MM) ----------------
        s1ps = psS.tile([b, jd], FP32, tag="sacc", name="s1ps")
        xgs = {}
        for g in range(groups):
            if g % 8 == 0:
                xc = xp.tile([c, 8 * p_i * b], FP16, tag="xg")
                nc.sync.dma_start(xc[:], xT[g // 8, :, :])
                xgs = {"t": xc, "g0": g}
            xg = xgs["t"]
            xoff = (g - xgs["g0"]) * p_i * b
            wg = wp.tile([c, p_i * jd], FP16, tag="wg")
            nc.sync.dma_start(wg[:], Wt[g, :, :])
            ug = u_tiles[g]
            for h in range(2):
                ps = psA.tile([128, half], FP32, tag="psA")
                for k in range(p_i):
                    for n in range(2):
                        n0, n1 = n * 512, (n + 1) * 512
                        c0 = k * jd + h * half
                        nc.tensor.matmul(
                            ps[k * b:(k + 1) * b, n0:n1],
                            lhsT=xg[:, xoff + k * b:xoff + (k + 1) * b],
                            rhs=wg[:, c0 + n0:c0 + n1],
                            start=True, stop=True,
                            tile_position=(0, (k * b) % 128),
                        )
                nc.scalar.copy(out=ug[:, h * half:(h + 1) * half], in_=ps[:])
            for n in range(nch):
                n0, n1 = n * 512, (n + 1) * 512
                nc.tensor.matmul(s1ps[:, n0:n1], lhsT=dMb[:],
                                 rhs=ug[:, n0:n1],
                                 start=(g == 0), stop=(g == groups - 1))

        def all_reduce(src_ps, idx, scale):
            # src_ps: (b, jd) fp32 PSUM accumulator -> SBUF -> DRAM -> AR
            nc.scalar.mul(out=s_sb[:], in_=src_ps[:], mul=scale)
            nc.sync.dma_start(ar_in[:], s_sb[:])
            nc.gpsimd.collective_compute(
                "AllReduce", ALU.add,
                replica_groups=replica_groups,
                ins=[ar_in.opt()],
                outs=[ar_out[idx].opt()],
            )
            nc.sync.dma_start(s_red[:], ar_out[idx][:])
            return s_red

        def squash(s_t):
            # factor = n/(1+n^2), n = ||s[b,j,:]||; v = s * factor (fp16)
            # s can reach ~1e3, so square (s/64)^2 to stay in fp16 range:
            # n2' = n2/4096; n = 64*sqrt(n2'); factor = 32*(2*sqrt(n2'))/(1+4096*n2')
            sq = scrp.tile([b, jd], FP16, tag="scr")
            nc.scalar.activation(out=sq[:], in_=s_t[:], func=AF.Square,
                                 scale=1.0 / 64)
            nc.vector.tensor_reduce(
                out=n2[:, :, None],
                in_=sq[:].rearrange("p (j d) -> p j d", j=j),
                axis=mybir.AxisListType.X, op=ALU.add)
            nc.scalar.activation(out=r0[:], in_=n2[:], func=AF.Sqrt)
            nc.vector.reciprocal(out=rr[:], in_=r0[:])
            nc.vector.tensor_tensor(out=rr[:], in0=rr[:], in1=n2[:], op=ALU.mult)
            nc.vector.tensor_tensor(out=rr[:], in0=rr[:], in1=r0[:], op=ALU.add)
            nc.vector.tensor_scalar_mul(dn[:], n2[:], 4096.0)
            nc.vector.tensor_scalar_add(dn[:], dn[:], 1.0)
            nc.vector.reciprocal(out=dn[:], in_=dn[:])
            nc.vector.tensor_tensor(out=dn[:], in0=dn[:], in1=rr[:], op=ALU.mult)
            nc.vector.tensor_scalar_mul(dn[:], dn[:], 32.0)
            nc.vector.tensor_tensor(
                out=v_t[:].rearrange("p (j d) -> p j d", j=j),
                in0=s_t[:].rearrange("p (j d) -> p j d", j=j),
                in1=dn[:, :, None].to_broadcast((b, j, d)),
                op=ALU.mult)
            return v_t

        def replicate(v):
            # v (b, jd) fp16 -> vrep (128, jd) fp16 via delta matmul
            for h in range(2):
                psr = psA.tile([128, half], FP32, tag="psA")
                for n in range(2):
                    n0, n1 = n * 512, (n + 1) * 512
                    nc.tensor.matmul(psr[:, n0:n1], lhsT=dbM[:],
                                     rhs=v[:, h * half + n0:h * half + n1],
                                     start=True, stop=True)
                nc.scalar.copy(out=vrep[:, h * half:(h + 1) * half], in_=psr[:])
            return vrep

        dbg = dims.get("DEBUG")

        s_r = all_reduce(s1ps, 0, 1.0 / j)
        if dims.get("ONLY_A"):
            nc.scalar.copy(out=v_t[:], in_=s_r[:])
            nc.sync.dma_start(out[:], v_t[:])
            return
        v = squash(s_r)
        if dbg:
            nc.sync.dma_start(outs["dbg_s1"][:], s_red[:])
            nc.sync.dma_start(outs["dbg_v1"][:], v[:])
            nc.sync.dma_start(outs["dbg_u0"][:], u_tiles[0][:])
        vr = replicate(v)

        # ---------------- Routing passes 2 and 3 ----------------
        # Software-pipelined blocks of BG groups: emission order
        # A(0) S(0) A(1) W(0) S(1) A(2) W(1) ... so DVE runs block b+1's
        # agreement while ACT expands block b's c and PE runs its delta-MMs.
        BG = dims.get("BG", 8)
        nblk = groups // BG

        for it in range(2):
            btile = bstate if it == 0 else b2
            # subtract target: reuse the other b tile (free at that point)
            subt = b2 if it == 0 else bstate
            sps = psS.tile([b, jd], FP32, tag="sacc", name=f"sps{it}")

            def stage_agree(bi):
                for g in range(bi * BG, (bi + 1) * BG):
                    t = scrp.tile([128, jd], FP16, tag="scr")
                    eng = (nc.gpsimd if (gps_agr and g % gps_agr == 1)
                           else nc.vector)
                    eng.tensor_tensor(out=t[:], in0=u_tiles[g][:], in1=vr[:],
                                      op=ALU.mult)
                    nc.vector.tensor_reduce(
                        out=btile[:, g * j:(g + 1) * j, None],
                        in_=t[:].rearrange("p (j d) -> p j d", j=j),
                        axis=mybir.AxisListType.X, op=ALU.add)

            def stage_softmax(bi):
                sl = slice(bi * BG * j, (bi + 1) * BG * j)
                slg = slice(bi * BG, (bi + 1) * BG)
                if it == 1:
                    nc.vector.tensor_tensor(out=b2[:, sl], in0=b2[:, sl],
                                            in1=bstate[:, sl], op=ALU.add)
                nc.vector.tensor_reduce(
                    out=nmx[:, slg, None],
                    in_=btile[:, sl].rearrange("p (g j) -> p g j", g=BG),
                    axis=mybir.AxisListType.X, op=ALU.max, negate=True)
                nc.vector.tensor_tensor(
                    out=subt[:, sl].rearrange("p (g j) -> p g j", g=BG),
                    in0=btile[:, sl].rearrange("p (g j) -> p g j", g=BG),
                    in1=nmx[:, slg, None].to_broadcast((128, BG, j)),
                    op=ALU.add)
                nc.scalar.activation(out=eb[:, sl], in_=subt[:, sl],
                                     func=AF.Exp)
                nc.vector.tensor_reduce(
                    out=zs[:, slg, None],
                    in_=eb[:, sl].rearrange("p (g j) -> p g j", g=BG),
                    axis=mybir.AxisListType.X, op=ALU.add)
                nc.vector.reciprocal(out=rz[:, slg], in_=zs[:, slg])

            def stage_w(bi):
                for g in range(bi * BG, (bi + 1) * BG):
                    ce = scrp.tile([128, jd], FP16, tag="scr")
                    ce_in = eb[:, g * j:(g + 1) * j, None].to_broadcast(
                        (128, j, d))
                    ce_out = ce[:].rearrange("p (j d) -> p j d", j=j)
                    if ce_eng == "act":
                        nc.scalar.mul(out=ce_out, in_=ce_in,
                                      mul=rz[:, g:g + 1])
                    else:
                        eng0 = (nc.gpsimd if ce_eng == "gpsimd"
                                else nc.vector)
                        eng0.tensor_scalar_mul(ce_out, ce_in, rz[:, g:g + 1])
                    w = scrp.tile([128, jd], FP16, tag="scr")
                    eng = (nc.gpsimd if (gps_w and g % gps_w == 0)
                           else nc.vector)
                    eng.tensor_tensor(out=w[:], in0=u_tiles[g][:], in1=ce[:],
                                      op=ALU.mult)
                    for n in range(nch):
                        n0, n1 = n * 512, (n + 1) * 512
                        nc.tensor.matmul(sps[:, n0:n1], lhsT=dMb[:],
                                         rhs=w[:, n0:n1],
                                         start=(g == 0),
                                         stop=(g == groups - 1))

            stage_agree(0)
            stage_softmax(0)
            for bi in range(1, nblk):
                stage_agree(bi)
                stage_w(bi - 1)
                stage_softmax(bi)
            stage_w(nblk - 1)

            s_r = all_reduce(sps, it + 1, 1.0)
            v = squash(s_r)
            if it == 0:
                vr = replicate(v)

        nc.sync.dma_start(out[:], v[:])


def _host_prep(x, W, n_cores, dims):
    """Shard + transpose inputs per core (fp16)."""
    b, j, d, c = dims["B"], dims["J"], dims["D"], dims["C"]
    i_core = dims["I_CORE"]
    p_i = 128 // b
    groups = i_core // p_i
    kc = n_cores
    jh = j // 2
    # xT: (kc, G/8, C, 8*P_I*B) from x (B, I, C)
    xt = np.ascontiguousarray(
        x.reshape(b, kc, groups // 8, 8, p_i, c).transpose(1, 2, 5, 3, 4, 0)
    ).reshape(kc, groups // 8, c, 8 * p_i * b).astype(np.float16)
    # Wt: (kc, G, C, P_I*JD) from W (J, I, D, C); cols = (k, j, d)
    wt = np.ascontiguousarray(
        W.reshape(j, kc, groups, p_i, d, c).transpose(1, 2, 5, 3, 0, 4)
    ).reshape(kc, groups, c, p_i * j * d).astype(np.float16)
    d_bM = np.tile(np.eye(b, dtype=np.float16), (1, p_i))      # (B, 128)
    d_Mb = np.ascontiguousarray(d_bM.T)                        # (128, B)
    in_maps = []
    for k in range(kc):
        in_maps.append({"xT": xt[k], "Wt": wt[k], "d_bM": d_bM, "d_Mb": d_Mb})
    return in_maps


def make_nc(dims):
    nc = bacc.Bacc("TRN2", target_bir_lowering=False, debug=False,
                   enable_asserts=False, num_devices=dims["N_CORES"])
    b, j, d, c = dims["B"], dims["J"], dims["D"], dims["C"]
    p_i = 128 // b
    groups = dims["I_CORE"] // p_i
    ins = {
        "xT": nc.dram_tensor("xT", [groups // 8, c, 8 * p_i * b], FP16,
                             kind="ExternalInput").ap(),
        "Wt": nc.dram_tensor("Wt", [groups, c, p_i * j * d], FP16,
                             kind="ExternalInput").ap(),
        "d_bM": nc.dram_tensor("d_bM", [b, 128], FP16,
                               kind="ExternalInput").ap(),
        "d_Mb": nc.dram_tensor("d_Mb", [128, b], FP16,
                               kind="ExternalInput").ap(),
    }
    outs = {
        "out": nc.dram_tensor("out", [b, j * d], FP16,
                              kind="ExternalOutput").ap(),
    }
    if dims.get("DEBUG"):
        for nm, shape, dt in [
            ("dbg_s1", [b, j * d], FP16), ("dbg_v1", [b, j * d], FP16),
            ("dbg_u0", [128, j * d], FP16),
        ]:
            outs[nm] = nc.dram_tensor(nm, shape, dt,
                                      kind="ExternalOutput").ap()
    with tile.TileContext(nc) as tc:
        build_kernel(nc, tc, dims, ins, outs)
    nc.compile()
    return nc


_NC_CACHE = {}


def _build_runner(nc, n_cores):
    """Mirror of bass2jax.run_bass_via_pjrt multi-core tail, returning the
    jitted sharded callable so callers can re-invoke with device-resident
    inputs for timing."""
    import jax
    from jax.sharding import Mesh, PartitionSpec
    from jax.experimental.shard_map import shard_map
    import concourse.mybir as mb
    from concourse.bass2jax import (_bass_exec_p, install_neuronx_cc_hook,
                                    partition_id_tensor)
    install_neuronx_cc_hook()
    partition_name = (nc.partition_id_tensor.name
                      if nc.partition_id_tensor else None)
    in_names, out_names, out_avals, zero_outs = [], [], [], []
    for alloc in nc.m.functions[0].allocations:
        if not isinstance(alloc, mb.MemoryLocationSet):
            continue
        name = alloc.memorylocations[0].name
        if alloc.kind == "ExternalInput":
            if name != partition_name:
                in_names.append(name)
        elif alloc.kind == "ExternalOutput":
            shape = tuple(alloc.tensor_shape)
            dtype = mb.dt.np(alloc.dtype)
            out_avals.append(jax.core.ShapedArray(shape, dtype))
            zero_outs.append(np.zeros(shape, dtype))
            out_names.append(name)
    n_params = len(in_names)
    n_outs = len(out_avals)
    all_in_names = list(in_names) + list(out_names)
    if partition_name is not None:
        all_in_names.append(partition_name)
    donate = tuple(range(n_params, n_params + n_outs))

    def _body(*args):
        operands = list(args)
        if partition_name is not None:
            operands.append(partition_id_tensor())
        return tuple(_bass_exec_p.bind(
            *operands, out_avals=tuple(out_avals), in_names=tuple(all_in_names),
            out_names=tuple(out_names), lowering_input_output_aliases=(),
            sim_require_finite=True, sim_require_nnan=True, nc=nc))

    devices = jax.devices()[:n_cores]
    mesh = Mesh(np.asarray(devices), ("core",))
    in_specs = (PartitionSpec("core"),) * (n_params + n_outs)
    out_specs = (PartitionSpec("core"),) * n_outs
    fn = jax.jit(shard_map(_body, mesh=mesh, in_specs=in_specs,
                           out_specs=out_specs, check_rep=False),
                 donate_argnums=donate, keep_unused=True)
    return {"fn": fn, "in_names": in_names, "out_names": out_names,
            "out_avals": out_avals, "zero_outs": zero_outs, "mesh": mesh,
            "n_params": n_params}


EXTRA_DIMS = {}


def _get_runner():
    dims = {"B": B, "J": J, "D": D, "C": C, "I_CORE": I // N_CORES,
            "N_CORES": N_CORES}
    dims.update(EXTRA_DIMS)
    if "full" not in _NC_CACHE:
        nc = make_nc(dims)
        _NC_CACHE["full"] = (nc, _build_runner(nc, N_CORES), dims)
    return _NC_CACHE["full"]


def _concat_inputs(runner, in_maps, n_cores):
    return [np.concatenate([np.asarray(in_maps[c][name])
                            for c in range(n_cores)], axis=0)
            for name in runner["in_names"]]


def _concat_zeros(runner, n_cores):
    return [np.zeros((n_cores * z.shape[0], *z.shape[1:]), z.dtype)
            for z in runner["zero_outs"]]


def kernel(x, W):
    nc, runner, dims = _get_runner()
    in_maps = _host_prep(np.asarray(x, np.float32), np.asarray(W, np.float32),
                         N_CORES, dims)
    concat_in = _concat_inputs(runner, in_maps, N_CORES)
    out_arrs = runner["fn"](*concat_in, *_concat_zeros(runner, N_CORES))
    idx = runner["out_names"].index("out")
    aval = runner["out_avals"][idx]
    out = np.asarray(out_arrs[idx]).reshape(N_CORES, *aval.shape)[0]
    return out.reshape(B, J, D).astype(np.float32)


# revision 19
# speedup vs baseline: 3.0254x; 1.3106x over previous
"""CapsuleLayer (dynamic routing) Trainium2 kernel.

Sharding: in_units I=1024 split across 8 cores (128 each); W sharded on I;
the per-iteration s_j = sum_i c_ij*u_hat reduction is completed with a
fp16 AllReduce (128 KB) per routing iteration.

Design:
  - Single fp16 for x and W (tolerance 2e-2; fp16 end-to-end err ~5.6e-3;
    bf16/fp8 fail — routing amplifies u_hat quantization ~6x). Halves HBM
    traffic (67 MB W per core streams at the DMA roofline, ~190 us).
  - u_hat resident in SBUF as fp16 (16.8 MB/core = 128 KiB/partition): no
    DRAM spill; routing passes do zero HBM traffic.
  - Phase A: per group tile (p=4 i's x b=32, free=(j,d)=2048), one fp16
    matmul per (i, 512-chunk) packed 4-up via tile_position col strips;
    PSUM fp32 -> fp16 SBUF copies on ScalarE; s1 accumulated with delta
    matmuls (128->32 partition fold) into a persistent PSUM tile.
  - Routing passes software-pipelined in blocks of 8 groups
    (A(b+1) emitted before W(b)): DVE does agreement mult (2x fp16) +
    segmented tensor_reduce (the 1x bottleneck) + c*u mult; GpSimd takes
    half the agreement mults; ScalarE expands c (broadcast AP * 1/Z
    per-partition scale) and evacuates PSUM; PE runs the delta-MM chain.
  - squash squares s/64 to stay in fp16 range (|s| reaches ~700).
"""

import numpy as np

import concourse.bass as bass
import concourse.bacc as bacc
import concourse.mybir as mybir
from concourse import tile

AF = mybir.ActivationFunctionType
ALU = mybir.AluOpType
FP32 = mybir.dt.float32
FP16 = mybir.dt.float16

# Full-problem dims
B, I, C = 32, 1024, 128
J, D = 32, 64
N_CORES = 8


def build_kernel(nc, tc, dims, ins, outs):
    b, j, d, c = dims["B"], dims["J"], dims["D"], dims["C"]
    i_core = dims["I_CORE"]
    jd = j * d
    p_i = 128 // b                 # i's packed per 128-partition tile (4)
    groups = i_core // p_i         # group tiles per core (32)
    half = jd // 2                 # 1024
    nch = jd // 512                # fp32-out matmul chunks (4)
    replica_groups = [list(range(dims["N_CORES"]))]

    gps_agr = dims.get("GPS_AGR", 2)   # every nth group's agreement mult on GpSimd
    gps_w = dims.get("GPS_W", 0)       # every nth group's c*u mult on GpSimd
    ce_eng = dims.get("CE_ENG", "act")

    xT, Wt = ins["xT"], ins["Wt"]      # (G, C, P_I*B), (G, 2, C, P_I*JD/2)
    d_bM, d_Mb = ins["d_bM"], ins["d_Mb"]
    out = outs["out"]                  # (B, JD) fp16

    with (
        tc.tile_pool(name="const", bufs=1) as constp,
        tc.tile_pool(name="w", bufs=dims.get("WB", 2)) as wp,
        tc.tile_pool(name="x", bufs=dims.get("XB", 1)) as xp,
        tc.tile_pool(name="u", bufs=1) as up,
        tc.tile_pool(name="scr", bufs=dims.get("SCRB", 5)) as scrp,
        tc.tile_pool(name="big", bufs=1) as bigp,
        tc.tile_pool(name="small", bufs=1) as smp,
        tc.tile_pool(name="sq", bufs=1) as sqp,
        tc.tile_pool(name="psA", bufs=2, space="PSUM") as psA,
        tc.tile_pool(name="psS", bufs=1, space="PSUM") as psS,
        tc.tile_pool(name="dram", bufs=1, space="DRAM") as dram,
    ):
        dbM = constp.tile([b, 128], FP16)
        dMb = constp.tile([128, b], FP16)
        nc.sync.dma_start(dbM[:], d_bM[:])
        nc.sync.dma_start(dMb[:], d_Mb[:])

        ar_in = dram.tile([b, jd], FP16)
        ar_out = [dram.tile([b, jd], FP16, tag=f"ar_out{i}", name=f"ar_out{i}")
                  for i in range(3)]

        u_tiles = [up.tile([128, jd], FP16, tag=f"u{g}", name=f"u{g}")
                   for g in range(groups)]
        # bstate holds (g-major, j) agreement sums: shape (128, groups*j)
        bstate = bigp.tile([128, groups * j], FP32, tag="bst", name="bst")
        b2 = bigp.tile([128, groups * j], FP32, tag="b2", name="b2")
        eb = bigp.tile([128, groups * j], FP16, tag="eb", name="eb")
        vrep = bigp.tile([128, jd], FP16, tag="vrep", name="vrep")
        nmx = smp.tile([128, groups], FP32, tag="nmx")
        zs = smp.tile([128, groups], FP32, tag="zs")
        rz = smp.tile([128, groups], FP32, tag="rz")
        s_sb = bigp.tile([b, jd], FP16, tag="s_sb", name="s_sb")
        s_red = bigp.tile([b, jd], FP16, tag="s_red", name="s_red")
        n2 = sqp.tile([b, j], FP32, tag="n2")
        r0 = sqp.tile([b, j], FP32, tag="r0")
        rr = sqp.tile([b, j], FP32, tag="rr")
        dn = sqp.tile([b, j], FP32, tag="dn")

        # ---------------- Phase A: u_hat (+ s1 delta-MM) ----------------
        s1ps = psS.tile([b, jd], FP32, tag="sacc", name="s1ps")
        xgs = {}
        for g in range(groups):
            if g % 8 == 0:
                xc = xp.tile([c, 8 * p_i * b], FP16, tag="xg")
                nc.sync.dma_start(xc[:], xT[g // 8, :, :])
                xgs = {"t": xc, "g0": g}
            xg = xgs["t"]
            xoff = (g - xgs["g0"]) * p_i * b
            wg = wp.tile([c, p_i * jd], FP16, tag="wg")
            nc.sync.dma_start(wg[:], Wt[g, :, :])
            ug = u_tiles[g]
            for h in range(2):
                ps = psA.tile([128, half], FP32, tag="psA")
                for k in range(p_i):
                    for n in range(2):
                        n0, n1 = n * 512, (n + 1) * 512
                        c0 = k * jd + h * half
                        nc.tensor.matmul(
                            ps[k * b:(k + 1) * b, n0:n1],
                            lhsT=xg[:, xoff + k * b:xoff + (k + 1) * b],
                            rhs=wg[:, c0 + n0:c0 + n1],
                            start=True, stop=True,
                            tile_position=(0, (k * b) % 128),
                        )
                nc.scalar.copy(out=ug[:, h * half:(h + 1) * half], in_=ps[:])
            for n in range(nch):
                n0, n1 = n * 512, (n + 1) * 512
                nc.tensor.matmul(s1ps[:, n0:n1], lhsT=dMb[:],
                                 rhs=ug[:, n0:n1],
                                 start=(g == 0), stop=(g == groups - 1))

        def all_reduce(src_ps, idx, scale):
            # src_ps: (b, jd) fp32 PSUM accumulator -> SBUF -> DRAM -> AR
            nc.scalar.mul(out=s_sb[:], in_=src_ps[:], mul=scale)
            nc.sync.dma_start(ar_in[:], s_sb[:])
            nc.gpsimd.collective_compute(
                "AllReduce", ALU.add,
                replica_groups=replica_groups,
                ins=[ar_in.opt()],
                outs=[ar_out[idx].opt()],
            )
            nc.sync.dma_start(s_red[:], ar_out[idx][:])
            return s_red

        def squash(s_t):
            # factor = n/(1+n^2), n = ||s[b,j,:]||; v = s * factor (fp16)
            # s can reach ~1e3, so square (s/64)^2 to stay in fp16 range:
            # n2' = n2/4096; n = 64*sqrt(n2'); factor = 32*(2*sqrt(n2'))/(1+4096*n2')
            v_t = scrp.tile([b, jd], FP16, tag="scr", name="v_sq")
            sq = scrp.tile([b, jd], FP16, tag="scr")
            nc.scalar.activation(out=sq[:], in_=s_t[:], func=AF.Square,
                                 scale=1.0 / 64)
            nc.vector.tensor_reduce(
                out=n2[:, :, None],
                in_=sq[:].rearrange("p (j d) -> p j d", j=j),
                axis=mybir.AxisListType.X, op=ALU.add)
            nc.scalar.activation(out=r0[:], in_=n2[:], func=AF.Sqrt)
            nc.vector.reciprocal(out=rr[:], in_=r0[:])
            nc.vector.tensor_tensor(out=rr[:], in0=rr[:], in1=n2[:], op=ALU.mult)
            nc.vector.tensor_tensor(out=rr[:], in0=rr[:], in1=r0[:], op=ALU.add)
            nc.vector.tensor_scalar_mul(dn[:], n2[:], 4096.0)
            nc.vector.tensor_scalar_add(dn[:], dn[:], 1.0)
            nc.vector.reciprocal(out=dn[:], in_=dn[:])
            nc.vector.tensor_tensor(out=dn[:], in0=dn[:], in1=rr[:], op=ALU.mult)
            nc.vector.tensor_scalar_mul(dn[:], dn[:], 32.0)
            nc.vector.tensor_tensor(
                out=v_t[:].rearrange("p (j d) -> p j d", j=j),
                in0=s_t[:].rearrange("p (j d) -> p j d", j=j),
                in1=dn[:, :, None].to_broadcast((b, j, d)),
                op=ALU.mult)
            return v_t

        def replicate(v):
            # v (b, jd) fp16 -> vrep (128, jd) fp16 via delta matmul
            for h in range(2):
                psr = psA.tile([128, half], FP32, tag="psA")
                for n in range(2):
                    n0, n1 = n * 512, (n + 1) * 512
                    nc.tensor.matmul(psr[:, n0:n1], lhsT=dbM[:],
                                     rhs=v[:, h * half + n0:h * half + n1],
                                     start=True, stop=True)
                nc.scalar.copy(out=vrep[:, h * half:(h + 1) * half], in_=psr[:])
            return vrep

        dbg = dims.get("DEBUG")

        s_r = all_reduce(s1ps, 0, 1.0 / j)
        if dims.get("ONLY_A"):
            nc.sync.dma_start(out[:], s_r[:])
            return
        v = squash(s_r)
        if dbg:
            nc.sync.dma_start(outs["dbg_s1"][:], s_red[:])
            nc.sync.dma_start(outs["dbg_v1"][:], v[:])
            nc.sync.dma_start(outs["dbg_u0"][:], u_tiles[0][:])
        vr = replicate(v)

        # ---------------- Routing passes 2 and 3 ----------------
        # Software-pipelined blocks of BG groups: emission order
        # A(0) S(0) A(1) W(0) S(1) A(2) W(1) ... so DVE runs block b+1's
        # agreement while ACT expands block b's c and PE runs its delta-MMs.
        BG = dims.get("BG", 8)
        nblk = groups // BG

        for it in range(2):
            btile = bstate if it == 0 else b2
            # subtract target: reuse the other b tile (free at that point)
            subt = b2 if it == 0 else bstate
            sps = psS.tile([b, jd], FP32, tag="sacc", name=f"sps{it}")

            def stage_agree(bi):
                for g in range(bi * BG, (bi + 1) * BG):
                    t = scrp.tile([128, jd], FP16, tag="scr")
                    eng = (nc.gpsimd if (gps_agr and g % gps_agr == 1)
                           else nc.vector)
                    eng.tensor_tensor(out=t[:], in0=u_tiles[g][:], in1=vr[:],
                                      op=ALU.mult)
                    nc.vector.tensor_reduce(
                        out=btile[:, g * j:(g + 1) * j, None],
                        in_=t[:].rearrange("p (j d) -> p j d", j=j),
                        axis=mybir.AxisListType.X, op=ALU.add)

            def stage_softmax(bi):
                sl = slice(bi * BG * j, (bi + 1) * BG * j)
                slg = slice(bi * BG, (bi + 1) * BG)
                if it == 1:
                    nc.vector.tensor_tensor(out=b2[:, sl], in0=b2[:, sl],
                                            in1=bstate[:, sl], op=ALU.add)
                nc.vector.tensor_reduce(
                    out=nmx[:, slg, None],
                    in_=btile[:, sl].rearrange("p (g j) -> p g j", g=BG),
                    axis=mybir.AxisListType.X, op=ALU.max, negate=True)
                nc.vector.tensor_tensor(
                    out=subt[:, sl].rearrange("p (g j) -> p g j", g=BG),
                    in0=btile[:, sl].rearrange("p (g j) -> p g j", g=BG),
                    in1=nmx[:, slg, None].to_broadcast((128, BG, j)),
                    op=ALU.add)
                nc.scalar.activation(out=eb[:, sl], in_=subt[:, sl],
                                     func=AF.Exp)
                nc.vector.tensor_reduce(
                    out=zs[:, slg, None],
                    in_=eb[:, sl].rearrange("p (g j) -> p g j", g=BG),
                    axis=mybir.AxisListType.X, op=ALU.add)
                nc.vector.reciprocal(out=rz[:, slg], in_=zs[:, slg])

            def stage_w(bi):
                for g in range(bi * BG, (bi + 1) * BG):
                    ce = scrp.tile([128, jd], FP16, tag="scr")
                    ce_in = eb[:, g * j:(g + 1) * j, None].to_broadcast(
                        (128, j, d))
                    ce_out = ce[:].rearrange("p (j d) -> p j d", j=j)
                    if ce_eng == "act":
                        nc.scalar.mul(out=ce_out, in_=ce_in,
                                      mul=rz[:, g:g + 1])
                    else:
                        eng0 = (nc.gpsimd if ce_eng == "gpsimd"
                                else nc.vector)
                        eng0.tensor_scalar_mul(ce_out, ce_in, rz[:, g:g + 1])
                    w = scrp.tile([128, jd], FP16, tag="scr")
                    eng = (nc.gpsimd if (gps_w and g % gps_w == 0)
                           else nc.vector)
                    eng.tensor_tensor(out=w[:], in0=u_tiles[g][:], in1=ce[:],
                                      op=ALU.mult)
                    for n in range(nch):
                        n0, n1 = n * 512, (n + 1) * 512
                        nc.tensor.matmul(sps[:, n0:n1], lhsT=dMb[:],
                                         rhs=w[:, n0:n1],
                                         start=(g == 0),
                                         stop=(g == groups - 1))

            stage_agree(0)
            stage_softmax(0)
            for bi in range(1, nblk):
                stage_agree(bi)
                stage_w(bi - 1)
                stage_softmax(bi)
            stage_w(nblk - 1)

            s_r = all_reduce(sps, it + 1, 1.0)
            v = squash(s_r)
            if it == 0:
                vr = replicate(v)

        nc.sync.dma_start(out[:], v[:])


def _host_prep(x, W, n_cores, dims):
    """Shard + transpose inputs per core (fp16)."""
    b, j, d, c = dims["B"], dims["J"], dims["D"], dims["C"]
    i_core = dims["I_CORE"]
    p_i = 128 // b
    groups = i_core // p_i
    kc = n_cores
    jh = j // 2
    # xT: (kc, G/8, C, 8*P_I*B) from x (B, I, C)
    xt = np.ascontiguousarray(
        x.reshape(b, kc, groups // 8, 8, p_i, c).transpose(1, 2, 5, 3, 4, 0)
    ).reshape(kc, groups // 8, c, 8 * p_i * b).astype(np.float16)
    # Wt: (kc, G, C, P_I*JD) from W (J, I, D, C); cols = (k, j, d)
    wt = np.ascontiguousarray(
        W.reshape(j, kc, groups, p_i, d, c).transpose(1, 2, 5, 3, 0, 4)
    ).reshape(kc, groups, c, p_i * j * d).astype(np.float16)
    d_bM = np.tile(np.eye(b, dtype=np.float16), (1, p_i))      # (B, 128)
    d_Mb = np.ascontiguousarray(d_bM.T)                        # (128, B)
    in_maps = []
    for k in range(kc):
        in_maps.append({"xT": xt[k], "Wt": wt[k], "d_bM": d_bM, "d_Mb": d_Mb})
    return in_maps


def make_nc(dims):
    nc = bacc.Bacc("TRN2", target_bir_lowering=False, debug=False,
                   enable_asserts=False, num_devices=dims["N_CORES"])
    b, j, d, c = dims["B"], dims["J"], dims["D"], dims["C"]
    p_i = 128 // b
    groups = dims["I_CORE"] // p_i
    ins = {
        "xT": nc.dram_tensor("xT", [groups // 8, c, 8 * p_i * b], FP16,
                             kind="ExternalInput").ap(),
        "Wt": nc.dram_tensor("Wt", [groups, c, p_i * j * d], FP16,
                             kind="ExternalInput").ap(),
        "d_bM": nc.dram_tensor("d_bM", [b, 128], FP16,
                               kind="ExternalInput").ap(),
        "d_Mb": nc.dram_tensor("d_Mb", [128, b], FP16,
                               kind="ExternalInput").ap(),
    }
    outs = {
        "out": nc.dram_tensor("out", [b, j * d], FP16,
                              kind="ExternalOutput").ap(),
    }
    if dims.get("DEBUG"):
        for nm, shape, dt in [
            ("dbg_s1", [b, j * d], FP16), ("dbg_v1", [b, j * d], FP16),
            ("dbg_u0", [128, j * d], FP16),
        ]:
            outs[nm] = nc.dram_tensor(nm, shape, dt,
                                      kind="ExternalOutput").ap()
    with tile.TileContext(nc) as tc:
        build_kernel(nc, tc, dims, ins, outs)
    nc.compile()
    return nc


_NC_CACHE = {}


def _build_runner(nc, n_cores):
    """Mirror of bass2jax.run_bass_via_pjrt multi-core tail, returning the
    jitted sharded callable so callers can re-invoke with device-resident
    inputs for timing."""
    import jax
    from jax.sharding import Mesh, PartitionSpec
    from jax.experimental.shard_map import shard_map
    import concourse.mybir as mb
    from concourse.bass2jax import (_bass_exec_p, install_neuronx_cc_hook,
                                    partition_id_tensor)
    install_neuronx_cc_hook()
    partition_name = (nc.partition_id_tensor.name
                      if nc.partition_id_tensor else None)
    in_names, out_names, out_avals, zero_outs = [], [], [], []
    for alloc in nc.m.functions[0].allocations:
        if not isinstance(alloc, mb.MemoryLocationSet):
            continue
        name = alloc.memorylocations[0].name
        if alloc.kind == "ExternalInput":
            if name != partition_name:
                in_names.append(name)
        elif alloc.kind == "ExternalOutput":
            shape = tuple(alloc.tensor_shape)
            dtype = mb.dt.np(alloc.dtype)
            out_avals.append(jax.core.ShapedArray(shape, dtype))
            zero_outs.append(np.zeros(shape, dtype))
            out_names.append(name)
    n_params = len(in_names)
    n_outs = len(out_avals)
    all_in_names = list(in_names) + list(out_names)
    if partition_name is not None:
        all_in_names.append(partition_name)
    donate = tuple(range(n_params, n_params + n_outs))

    def _body(*args):
        operands = list(args)
        if partition_name is not None:
            operands.append(partition_id_tensor())
        return tuple(_bass_exec_p.bind(
            *operands, out_avals=tuple(out_avals), in_names=tuple(all_in_names),
            out_names=tuple(out_names), lowering_input_output_aliases=(),
            sim_require_finite=True, sim_require_nnan=True, nc=nc))

    devices = jax.devices()[:n_cores]
    mesh = Mesh(np.asarray(devices), ("core",))
    in_specs = (PartitionSpec("core"),) * (n_params + n_outs)
    out_specs = (PartitionSpec("core"),) * n_outs
    fn = jax.jit(shard_map(_body, mesh=mesh, in_specs=in_specs,
                           out_specs=out_specs, check_rep=False),
                 donate_argnums=donate, keep_unused=True)
    return {"fn": fn, "in_names": in_names, "out_names": out_names,
            "out_avals": out_avals, "zero_outs": zero_outs, "mesh": mesh,
            "n_params": n_params}


EXTRA_DIMS = {}


def _get_runner():
    dims = {"B": B, "J": J, "D": D, "C": C, "I_CORE": I // N_CORES,
            "N_CORES": N_CORES}
    dims.update(EXTRA_DIMS)
    if "full" not in _NC_CACHE:
        nc = make_nc(dims)
        _NC_CACHE["full"] = (nc, _build_runner(nc, N_CORES), dims)
    return _NC_CACHE["full"]


def _concat_inputs(runner, in_maps, n_cores):
    return [np.concatenate([np.asarray(in_maps[c][name])
                            for c in range(n_cores)], axis=0)
            for name in runner["in_names"]]


def _concat_zeros(runner, n_cores):
    return [np.zeros((n_cores * z.shape[0], *z.shape[1:]), z.dtype)
            for z in runner["zero_outs"]]


def kernel(x, W):
    nc, runner, dims = _get_runner()
    in_maps = _host_prep(np.asarray(x, np.float32), np.asarray(W, np.float32),
                         N_CORES, dims)
    concat_in = _concat_inputs(runner, in_maps, N_CORES)
    out_arrs = runner["fn"](*concat_in, *_concat_zeros(runner, N_CORES))
    idx = runner["out_names"].index("out")
    aval = runner["out_avals"][idx]
    out = np.asarray(out_arrs[idx]).reshape(N_CORES, *aval.shape)[0]
    return out.reshape(B, J, D).astype(np.float32)
